# revision 1
# baseline (speedup 1.0000x reference)
import os
import sys

sys.path.insert(0, "/opt/trn_rl_repo")

import numpy as np

import concourse.bass as bass
import concourse.bacc as bacc_mod
import concourse.mybir as mybir
import concourse.tile as tile
from concourse.bass_utils import run_bass_kernel_spmd
from concourse.masks import make_identity

F32 = mybir.dt.float32
P = 128
NQ = 196
NK = 50
D = 512
H = 8
DI = 64
HD = H * DI  # 512
SCALE = float(np.sqrt(64.0) + 1e-6)
INV_SCALE = 1.0 / SCALE
NCORES = 8

# token-bank structure for batched softmax: 5 banks of <=40 tokens (20 pairs)
BANKS = [(40 * b, min(40, NQ - 40 * b)) for b in range(5)]

_CACHE = {}
LAST_RESULTS = None


def build_kernel():
    nc = bacc_mod.Bacc()
    q_d = nc.dram_tensor("queries", [NQ, D], F32, kind="ExternalInput")
    t_d = nc.dram_tensor("targets", [NQ, NK, D], F32, kind="ExternalInput")
    wq_d = nc.dram_tensor("Wq", [D, HD], F32, kind="ExternalInput")
    bq_d = nc.dram_tensor("bq", [HD], F32, kind="ExternalInput")
    wk_d = nc.dram_tensor("Wk", [D, HD], F32, kind="ExternalInput")
    bk_d = nc.dram_tensor("bk", [HD], F32, kind="ExternalInput")  # noqa: F841 (no effect on softmax)
    wv_d = nc.dram_tensor("Wv", [D, HD], F32, kind="ExternalInput")
    bv_d = nc.dram_tensor("bv", [HD], F32, kind="ExternalInput")
    wo_d = nc.dram_tensor("Wo", [HD, D], F32, kind="ExternalInput")
    bo_d = nc.dram_tensor("bo", [D], F32, kind="ExternalInput")
    out_d = nc.dram_tensor("out", [NQ, D], F32, kind="ExternalOutput")

    tflat = t_d.rearrange("t k d -> (t k) d")  # [9800, 512]
    tpair = tflat.rearrange("(g r) d -> g r d", r=100)  # [98, 100, 512]

    with tile.TileContext(nc) as tc:
        with (
            tc.tile_pool(name="const", bufs=1) as const,
            tc.tile_pool(name="nt", bufs=int(os.environ.get("NTBUFS", "3"))) as nt_pool,
            tc.tile_pool(name="tsb", bufs=int(os.environ.get("TSB", "4"))) as t_pool,
            tc.tile_pool(name="sm", bufs=int(os.environ.get("SMB", "2"))) as sm_pool,
            tc.tile_pool(name="attn", bufs=8) as attn_pool,
            tc.tile_pool(name="misc", bufs=3) as misc_pool,
            tc.tile_pool(name="ps_tr", bufs=2, space="PSUM") as ps_tr,
            tc.tile_pool(name="ps_att", bufs=2, space="PSUM") as ps_att,
            tc.tile_pool(name="ps_agg", bufs=1, space="PSUM") as ps_agg,
            tc.tile_pool(name="ps_gate", bufs=1, space="PSUM") as ps_gate,
            tc.tile_pool(name="ps_misc", bufs=2, space="PSUM") as ps_misc,
        ):
            ident = const.tile([P, P], F32)
            make_identity(nc, ident)

            # ---- load weights (natural: partition = row-inner, D-outer chunks) ----
            wq_sb = const.tile([P, 4, HD], F32)
            nc.sync.dma_start(wq_sb, wq_d.rearrange("(c p) n -> p c n", p=P))
            wk_sb = const.tile([P, 4, HD], F32)
            nc.sync.dma_start(wk_sb, wk_d.rearrange("(c p) n -> p c n", p=P))
            wv_sb = const.tile([P, 4, HD], F32)
            nc.sync.dma_start(wv_sb, wv_d.rearrange("(c p) n -> p c n", p=P))
            bq_sb = const.tile([P, 4], F32)
            nc.sync.dma_start(bq_sb, bq_d.rearrange("(c p) -> p c", p=P))
            bv_sb = const.tile([P, 4], F32)
            nc.sync.dma_start(bv_sb, bv_d.rearrange("(c p) -> p c", p=P))
            bo_bc = const.tile([P, D], F32)
            nc.gpsimd.dma_start(
                out=bo_bc,
                in_=bo_d.ap()[None, :].to_broadcast((P, D)),
            )

            # ---- WkT: transpose Wk -> [hd partitions (4 chunks), D free] ----
            # gate: plain matmul absorbs (DMA, ident) deps so transposes carry <=1 wait
            gate_ps = ps_gate.tile([P, 4], F32, tag="g")
            nc.tensor.matmul(gate_ps[0:2, 0:2], wk_sb[:, 0, 0:2], ident[:, 0:2], start=True, stop=True)
            wkT_sb = const.tile([P, 4, D], F32)
            for a in range(4):  # hd chunk (output partitions)
                ps = ps_misc.tile([P, 4, P], F32, tag="m")
                for bch in range(4):  # D chunk
                    nc.tensor.transpose(ps[:, bch, :], wk_sb[:, bch, a * P : (a + 1) * P], ident)
                nc.vector.tensor_copy(wkT_sb[:, a, :], ps.rearrange("p c f -> p (c f)"))

            # ---- queries^T ----
            qT_sb = const.tile([P, 4, NQ], F32)
            for tt in range(2):
                q_sb = misc_pool.tile([98, D], F32)
                nc.sync.dma_start(q_sb, q_d[tt * 98 : (tt + 1) * 98, :])
                gq = ps_gate.tile([P, 4], F32, tag="g")
                nc.tensor.matmul(gq[0:2, 0:2], q_sb[:, 0:2], ident[0:98, 0:2], start=True, stop=True)
                ps = ps_misc.tile([P, 4, 98], F32, tag="m")
                for c in range(4):
                    nc.tensor.transpose(ps[:, c, :], q_sb[:, c * P : (c + 1) * P], ident[0:98, 0:98])
                nc.vector.tensor_copy(qT_sb[:, :, tt * 98 : (tt + 1) * 98], ps)

            # ---- QT = (Wq^T @ queries^T + bq) * inv_scale : [hd, t] ----
            QT_sb = const.tile([P, 4, NQ], F32)
            for m in range(4):
                ps = ps_misc.tile([P, NQ], F32, tag="m")
                for c in range(4):
                    nc.tensor.matmul(
                        ps,
                        wq_sb[:, c, m * P : (m + 1) * P],
                        qT_sb[:, c, :],
                        start=(c == 0),
                        stop=(c == 3),
                    )
                nc.vector.tensor_scalar(
                    out=QT_sb[:, m, :],
                    in0=ps,
                    scalar1=bq_sb[:, m : m + 1],
                    scalar2=INV_SCALE,
                    op0=mybir.AluOpType.add,
                    op1=mybir.AluOpType.mult,
                )

            # ---- UT[D, t, h]: Kc=128 base-0 via zero-masked QT halves ----
            # QTz[0] has rows 64-127 zeroed (even heads), QTz[1] rows 0-63 zeroed
            QTz = []
            for par in range(2):
                qz = const.tile([P, 4, NQ], F32, tag=f"qtz{par}")
                nc.vector.memset(qz, 0.0)
                pb = par * 64
                nc.vector.tensor_copy(qz[pb : pb + 64, :, :], QT_sb[pb : pb + 64, :, :])
                QTz.append(qz)
            UT_sb = const.tile([P, 4, NQ, H], F32)
            TQ = [(0, 50), (50, 48), (98, 50), (148, 48)]
            for m in range(4):
                for q0, qn in TQ:
                    ps = ps_misc.tile([P, H, 50], F32, tag="m")
                    for h in range(H):
                        nc.tensor.matmul(
                            ps[:, h, 0:qn],
                            wkT_sb[:, h // 2, m * P : (m + 1) * P],
                            QTz[h % 2][:, h // 2, q0 : q0 + qn],
                            start=True,
                            stop=True,
                        )
                    nc.vector.tensor_copy(
                        UT_sb[:, m, q0 : q0 + qn, :],
                        ps[:, :, 0:qn].rearrange("p h t -> p t h"),
                    )

            # ---- AGGT accumulator in SBUF ----
            AGGT_sb = const.tile([P, 4, H, NQ], F32)

            # ---- stream over banks of 4 pairs (8 tokens) ----
            NPAIRS = NQ // 2  # 98
            NBANKS = (NPAIRS + 3) // 4  # 25
            NBANKS = min(NBANKS, int(os.environ.get("KBANKS", "25")))
            nt_tiles = {}
            ag_ps = None
            tok_ctr = 0
            for bk in range(NBANKS):
                npr = min(4, NPAIRS - 4 * bk)
                att_ps = ps_att.tile([16, 4, 2, 50], F32, tag="att")
                t_infos = []
                for jl in range(npr):
                    g = 4 * bk + jl
                    gg = g // 5
                    if gg not in nt_tiles:
                        cnt = min(5, NPAIRS - 5 * gg)
                        NTt = nt_pool.tile([64, 2, 5, D], F32)
                        nc.sync.dma_start(
                            NTt[0:50, 0, 0:cnt, :],
                            tpair[5 * gg : 5 * gg + cnt, 0:50, :].rearrange("g r d -> r g d"),
                        )
                        nc.sync.dma_start(
                            NTt[0:50, 1, 0:cnt, :],
                            tpair[5 * gg : 5 * gg + cnt, 50:100, :].rearrange("g r d -> r g d"),
                        )
                        ga = ps_gate.tile([P, 4], F32, tag="g")
                        nc.tensor.matmul(ga[0:2, 0:2], NTt[0:50, 0, 0, 0:2], ident[0:50, 0:2], start=True, stop=True)
                        gb = ps_gate.tile([P, 4], F32, tag="g")
                        nc.tensor.matmul(gb[0:2, 0:2], NTt[0:50, 1, 0, 0:2], ident[0:50, 0:2], start=True, stop=True)
                        nt_tiles[gg] = NTt
                    NTt = nt_tiles[gg]
                    jj = g % 5
                    psT = ps_tr.tile([P, 4, P], F32, tag="tr")
                    for c in range(4):
                        nc.tensor.transpose(
                            psT[:, c, 0:50], NTt[0:50, 0, jj, c * P : (c + 1) * P], ident[0:50, 0:50]
                        )
                        nc.tensor.transpose(
                            psT[:, c, 64:114], NTt[0:50, 1, jj, c * P : (c + 1) * P], ident[0:50, 0:50]
                        )
                    T_sb = t_pool.tile([P, 4, P], F32)
                    nc.vector.tensor_copy(T_sb[:, :, 0:50], psT[:, :, 0:50])
                    nc.vector.tensor_copy(T_sb[:, :, 64:114], psT[:, :, 64:114])
                    t_infos.append((NTt, jj))

                    tA = 2 * g
                    for c in range(4):
                        nc.tensor.matmul(
                            att_ps[:, jl, :, :],
                            UT_sb[:, c, tA : tA + 2, :],
                            T_sb[:, c, :].rearrange("p (s k) -> p s k", s=2)[:, :, 0:50],
                            start=(c == 0),
                            stop=(c == 3),
                        )

                # ---- softmax over k (free axis); strided view [16, npr, 2, 64->50] ----
                apv = att_ps[:, 0:npr, :, :]
                att_e = sm_pool.tile([16, 4, P], F32, tag="sme")
                aev = att_e[:, 0:npr, :].rearrange("p j (s k) -> p j s k", s=2)[:, :, :, 0:50]
                nc.scalar.activation(aev, apv, func=mybir.ActivationFunctionType.Exp)
                ssum = sm_pool.tile([16, 4, 2], F32, tag="sms")
                nc.vector.reduce_sum(ssum[:, 0:npr, :], aev, axis=mybir.AxisListType.X)
                sinv = sm_pool.tile([16, 4, 2], F32, tag="smi")
                nc.vector.reciprocal(sinv[:, 0:npr, :], ssum[:, 0:npr, :])
                att_n = sm_pool.tile([16, 4, P], F32, tag="smn")
                anv = att_n[:, 0:npr, :].rearrange("p j (s k) -> p j s k", s=2)[:, :, :, 0:50]
                nc.vector.tensor_tensor(
                    anv,
                    aev,
                    sinv[:, 0:npr, :, None].to_broadcast((16, npr, 2, 50)),
                    mybir.AluOpType.mult,
                )

                # ---- per pair: transpose att back to [k, (t,h)] and aggregate ----
                for jl in range(npr):
                    g = 4 * bk + jl
                    NTt, jj = t_infos[jl]
                    psn = ps_misc.tile([64, 2, 16], F32, tag="m")
                    nc.tensor.transpose(psn[0:50, 0, :], att_n[:, jl, 0:50], ident[0:16, 0:16])
                    nc.tensor.transpose(psn[0:50, 1, :], att_n[:, jl, 64:114], ident[0:16, 0:16])
                    a_sb = attn_pool.tile([64, 2, 16], F32)
                    nc.vector.tensor_copy(a_sb[0:50, :, :], psn[0:50, :, :])
                    for parity in range(2):
                        if tok_ctr % 16 == 0:
                            if ag_ps is not None:
                                i0 = tok_ctr - 16
                                nc.vector.tensor_copy(
                                    AGGT_sb[:, :, :, i0 : i0 + 16],
                                    ag_ps.rearrange("p t c h -> p c h t"),
                                )
                            ag_ps = ps_agg.tile([P, 16, 4, 8], F32, tag="agg")
                        tl = tok_ctr % 16
                        for c in range(4):
                            nc.tensor.matmul(
                                ag_ps[:, tl, c, :],
                                NTt[0:50, parity, jj, c * P : (c + 1) * P],
                                a_sb[0:50, parity, 8 * parity : 8 * parity + 8],
                                start=(c == 0),
                                stop=(c == 3),
                            )
                        tok_ctr += 1
            # flush last agg group
            i0 = (tok_ctr - 1) // 16 * 16
            rem = tok_ctr - i0
            nc.vector.tensor_copy(
                AGGT_sb[:, :, :, i0 : i0 + rem],
                ag_ps[:, 0:rem, :, :].rearrange("p t c h -> p c h t"),
            )

            # ---- V-step: Y^T in [64, (m,hh), t] chunks, psum base 0 ----
            YT_sb = const.tile([64, 4, 2, NQ], F32)
            for m in range(4):
                for hh in range(2):
                    h = 2 * m + hh
                    for tt in range(2):
                        ps = ps_misc.tile([64, 98], F32, tag="m")
                        for c in range(4):
                            nc.tensor.matmul(
                                ps,
                                wv_sb[:, c, m * P + hh * 64 : m * P + (hh + 1) * 64],
                                AGGT_sb[:, c, h, tt * 98 : (tt + 1) * 98],
                                start=(c == 0),
                                stop=(c == 3),
                            )
                        nc.vector.tensor_scalar(
                            out=YT_sb[:, m, hh, tt * 98 : (tt + 1) * 98],
                            in0=ps,
                            scalar1=bv_sb[hh * 64 : (hh + 1) * 64, m : m + 1],
                            scalar2=None,
                            op0=mybir.AluOpType.add,
                        )

            # ---- final: out = Y @ Wo + bo (8 x Kc=64 matmuls per token tile) ----
            wo_v = wo_d.rearrange("(m hh p) n -> p m hh n", p=64, hh=2)
            wo64_sb = const.tile([64, 4, 2, D], F32)
            nc.sync.dma_start(wo64_sb, wo_v)
            for tt in range(2):
                ps = ps_misc.tile([98, D], F32, tag="m")
                k_i = 0
                for m in range(4):
                    for hh in range(2):
                        nc.tensor.matmul(
                            ps,
                            YT_sb[:, m, hh, tt * 98 : (tt + 1) * 98],
                            wo64_sb[:, m, hh, :],
                            start=(k_i == 0),
                            stop=(k_i == 7),
                        )
                        k_i += 1
                o_sb = misc_pool.tile([98, D], F32)
                nc.vector.tensor_tensor(o_sb, ps, bo_bc[0:98, :], mybir.AluOpType.add)
                nc.sync.dma_start(out_d[tt * 98 : (tt + 1) * 98, :], o_sb)

    nc.compile()
    return nc


def kernel(**inputs):
    global LAST_RESULTS
    if "nc" not in _CACHE:
        _CACHE["nc"] = build_kernel()
    nc = _CACHE["nc"]

    queries = np.asarray(inputs["queries"], dtype=np.float32)
    targets = np.asarray(inputs["targets"], dtype=np.float32)
    shared = {
        k: np.ascontiguousarray(np.asarray(inputs[k], dtype=np.float32))
        for k in ("Wq", "bq", "Wk", "bk", "Wv", "bv", "Wo", "bo")
    }
    in_maps = []
    for i in range(NCORES):
        m = {"queries": np.ascontiguousarray(queries[i]), "targets": np.ascontiguousarray(targets[i])}
        m.update(shared)
        in_maps.append(m)

    res = run_bass_kernel_spmd(nc, in_maps, core_ids=list(range(NCORES)))
    LAST_RESULTS = res
    out = np.stack([res.results[i]["out"] for i in range(NCORES)], axis=0)
    return out

